# revision 12
# baseline (speedup 1.0000x reference)
"""Trainium2 Bass kernel for AttentionSocialPooling.

Strategy (8 cores, data parallel over batch B=8; core m handles batch b=m).
Per (b,t): score s[i,j] = sum_a w2_a*relu(u_a[i]+v_a[j]) + b2 with
u = pos@(W1p-W1d)+b1, v = pos@W1d.  One fp16 PE matmul per t materializes
c_a = e_a*(u_a+v_a) for all (i,a) columns (e_a = +/-w2_a, see below) via the
delta trick (lhsT rows [1; v], moving rows [u; delta]).

Channels are split into halves A (cols 0:8, value = +contribution) and
B (cols 8:16, value = -contribution); emission sign e_a = +w2 for A, -w2 for
B.  ACT evacuates PSUM->SBUF with Relu (A: pos-w2 channels; B: neg-w2
channels, whose negated emission makes Relu give |w2|relu = -contribution).
DVE evacuates the rest with tensor_scalar max/min (sign-flexible).  The
A-reduction is then a bf16 subtract-then-add tree, batched over 8 t's to
amortize the ~280ns DVE drain: L1 (A-B) and L2 on DVE at 2x mode, L3/L4 on
GPSIMD.  Sigmoid per 8 t's on ACT.

Distance mask softened: the dist matmul (fp16 hi/lo) emits
z = KAPPA*(R^2-d^2) and m = clip(z,0,1) in one DVE tensor_scalar.  The
diagonal stays in (m[i,i]=1): its numerator contribution cancels exactly
because pos_j and pos_i use the same fp16-rounded positions, and the count
subtracts 1 in the tail.  Final row sums via PE matmuls with w^T / m^T as
stationary; tail divides by count on DVE.
"""

import numpy as np
import ml_dtypes

B, T, N, C, A = 8, 64, 128, 2, 16
R2 = 2500.0
KAPPA = 16.0
NA = N * A             # 2048 columns per t
HALF = NA // 2
TG = 8                 # t-group size for tree/sigmoid batching

bf16 = ml_dtypes.bfloat16
f16 = np.float16

_CACHE = {}


def _plan_channels(w2):
    """Assign 16 channels to halves/engines.

    Returns (order, emis, plan) where order[slot] = original channel index,
    emis[slot] = emission coefficient, and plan describes evacuation slices:
    ACT slice [0, ka) spans A-ACT plus B-ACT (contiguous), then DVE slices.
    Slot layout: [A-ACT pos | A-DVE(max) pos | A-DVE(min) neg ||
                  B-ACT neg | B-DVE(max) neg | B-DVE(min) pos]
    For the common case (npos>=8, nneg<=8) the A half is all-ACT so the ACT
    region [0:8+nBA) is contiguous.
    """
    pos = [int(i) for i in np.where(w2 >= 0)[0]]
    neg = [int(i) for i in np.where(w2 < 0)[0]]
    npos, nneg = len(pos), len(neg)
    assert npos + nneg == A

    # B half: prefer negs; overflow pos go to B-DVE(min). A half: the rest.
    nBneg = min(nneg, 8)
    nBpos = 8 - nBneg
    nAneg = nneg - nBneg
    nApos = 8 - nAneg

    # how many B-negs ACT takes (balance knob)
    nBA = min(3, nBneg) if nApos == 8 else max(0, min(nBneg, 11 - nApos))

    a_act = pos[:nApos] if nApos <= npos else pos
    a_dve_max = []
    a_dve_min = neg[nBneg:]
    b_act = neg[:nBA]
    b_dve_max = neg[nBA:nBneg]
    b_dve_min = pos[nApos:]
    # A-ACT only valid for pos channels; if fewer pos than nApos we'd need
    # A-DVE(min) negs to fill: handled via a_dve_min above.
    order = a_act + a_dve_max + a_dve_min + b_act + b_dve_max + b_dve_min
    assert len(order) == A and sorted(order) == list(range(A))

    emis = np.empty(A, np.float32)
    for s, ch in enumerate(order):
        emis[s] = w2[ch] if s < 8 else -w2[ch]

    plan = dict(
        ka=len(a_act) + (len(b_act) if len(a_dve_max) == len(a_dve_min) == 0
                         else 0),
        a_act=len(a_act), a_max=len(a_dve_max), a_min=len(a_dve_min),
        b_act=len(b_act), b_max=len(b_dve_max), b_min=len(b_dve_min),
    )
    return order, emis, plan


def _host_prep(positions, W1, b1, W2, b2):
    pos = np.asarray(positions, dtype=np.float32)
    W1 = np.asarray(W1, dtype=np.float32)
    b1 = np.asarray(b1, dtype=np.float32)
    W2 = np.asarray(W2, dtype=np.float32)
    b2 = np.asarray(b2, dtype=np.float32)

    W1p, W1d = W1[:C], W1[C:]
    w2 = W2[:, 0]
    order, emis, plan = _plan_channels(w2)

    Wu = (W1p - W1d)[:, order] * emis
    Wd = W1d[:, order] * emis
    b1v = b1[order] * emis

    u = (pos @ Wu + b1v).astype(f16)     # [B,T,N,A]
    v = (pos @ Wd).astype(f16)

    vT = np.empty((B, 1 + A, T * N), dtype=f16)
    vT[:, 0] = np.asarray(1.0, dtype=f16)
    vT[:, 1:] = v.transpose(0, 3, 1, 2).reshape(B, A, T * N)

    uflat = np.ascontiguousarray(u.reshape(B, T, 1, NA))

    delta = np.zeros((A, NA), dtype=f16)
    for a in range(A):
        delta[a, a::A] = np.asarray(1.0, dtype=f16)

    # soft-mask matmul operands: z = KAPPA*(R2 - d2), sqrt(KAPPA) per side
    sk = np.sqrt(KAPPA)
    pos64 = pos.astype(np.float64)
    n2 = (pos64 ** 2).sum(-1)
    px = pos64[..., 0].reshape(B, T * N)
    py = pos64[..., 1].reshape(B, T * N)
    n2f = n2.reshape(B, T * N)

    def hilo(x):
        hi = x.astype(f16)
        lo = (x - hi.astype(np.float64)).astype(f16)
        return hi, lo

    pxh, pxl = hilo(sk * px)
    pyh, pyl = hilo(sk * py)
    n2jh, n2jl = hilo(-sk * n2f)
    p2xh, p2xl = hilo(2 * sk * px)
    p2yh, p2yl = hilo(2 * sk * py)
    n2ih, n2il = hilo(sk * (R2 - n2f))
    skones = np.full_like(pxh, sk)
    lhsTd = np.stack([pxh, pxh, pxl, pyh, pyh, pyl, skones, skones,
                      n2jh, n2jl], axis=1).astype(f16)
    rhsd = np.stack([p2xh, p2xl, p2xh, p2yh, p2yl, p2yh, n2ih, n2il,
                     skones, skones], axis=1).astype(f16)

    pos16 = pos.astype(f16)
    pos3 = np.empty((B, N, T * 3), f16)
    p3 = pos3.reshape(B, N, T, 3)
    p3[..., 0] = pos16[..., 0].transpose(0, 2, 1)
    p3[..., 1] = pos16[..., 1].transpose(0, 2, 1)
    p3[..., 2] = 1.0

    posI = np.empty((B, N, T * 2), np.float32)
    pI = posI.reshape(B, N, T, 2)
    pI[..., 0] = pos16[..., 0].astype(np.float32).transpose(0, 2, 1)
    pI[..., 1] = pos16[..., 1].astype(np.float32).transpose(0, 2, 1)

    return dict(vT=vT, uflat=uflat, delta=delta, lhsTd=lhsTd, rhsd=rhsd,
                pos3=pos3, posI=posI, plan=plan, b2=float(b2[0]))


def _build_program(plan_key, b2val, debug=False):
    import concourse.bacc as bacc
    import concourse.mybir as mybir
    import concourse.tile as tile

    f32 = mybir.dt.float32
    fp16 = mybir.dt.float16
    bfl = mybir.dt.bfloat16
    Alu = mybir.AluOpType
    Act = mybir.ActivationFunctionType

    (a_act, a_max, a_min, b_act, b_max, b_min) = plan_key
    K2 = 1 + A

    nc = bacc.Bacc()

    vT_p = nc.declare_dram_parameter("vT", [K2, T * N], fp16, isOutput=False)
    uflat_p = nc.declare_dram_parameter("uflat", [T, 1, NA], fp16, isOutput=False)
    delta_p = nc.declare_dram_parameter("delta", [A, NA], fp16, isOutput=False)
    lhsTd_p = nc.declare_dram_parameter("lhsTd", [10, T * N], fp16, isOutput=False)
    rhsd_p = nc.declare_dram_parameter("rhsd", [10, T * N], fp16, isOutput=False)
    pos3_p = nc.declare_dram_parameter("pos3", [N, T * 3], fp16, isOutput=False)
    posI_p = nc.declare_dram_parameter("posI", [N, T * 2], f32, isOutput=False)
    out_p = nc.declare_dram_parameter("out", [T, N, C], f32, isOutput=True)
    if debug:
        dbg_m = nc.declare_dram_parameter("dbg_m", [N, TG * N], fp16,
                                          isOutput=True)
        dbg_att = nc.declare_dram_parameter("dbg_att", [N, TG * N], fp16,
                                            isOutput=True)
        dbg_s4 = nc.declare_dram_parameter("dbg_s4", [N, TG * N], bfl,
                                           isOutput=True)
        dbg_r = nc.declare_dram_parameter("dbg_r", [N, NA], bfl,
                                          isOutput=True)

    with tile.TileContext(nc) as tc:
        with (
            tc.tile_pool(name="pers", bufs=1) as pers,
            tc.tile_pool(name="hpsum", bufs=3, space="PSUM") as hpsum,
            tc.tile_pool(name="dpsum", bufs=1, space="PSUM") as dpsum,
            tc.tile_pool(name="fpsum", bufs=1, space="PSUM") as fpsum,
            tc.tile_pool(name="rwork", bufs=2) as rwork,
            tc.tile_pool(name="swork", bufs=2) as swork,
            tc.tile_pool(name="awork", bufs=2) as awork,
            tc.tile_pool(name="twork", bufs=2) as twork,
        ):
            vT_s = pers.tile([K2, T * N], fp16, tag="vT")
            lhsTd_s = pers.tile([10, T * N], fp16, tag="lhsTd")
            rhsd_s = pers.tile([10, T * N], fp16, tag="rhsd")
            pos3_s = pers.tile([N, T * 3], fp16, tag="pos3")
            posI_s = pers.tile([N, T * 2], f32, tag="posI")
            rhH = [pers.tile([K2, NA], fp16, tag=f"rh{i}", name=f"rh{i}")
                   for i in range(4)]

            nc.gpsimd.dma_start(vT_s[:], vT_p[:])
            nc.gpsimd.dma_start(lhsTd_s[:], lhsTd_p[:])
            nc.gpsimd.dma_start(rhsd_s[:], rhsd_p[:])
            nc.gpsimd.dma_start(pos3_s[:], pos3_p[:])
            nc.gpsimd.dma_start(posI_s[:], posI_p[:])
            for i in range(4):
                nc.gpsimd.dma_start(rhH[i][1:K2, :], delta_p[:])

            pd = None
            R8 = None
            att8 = None
            m8 = None
            w8 = None
            pf = None
            for t in range(T):
                g2 = t % 2
                g8 = t % TG
                rh = rhH[t % 4]
                nc.sync.dma_start(rh[0:1, :], uflat_p[t])

                hpA = hpsum.tile([N, HALF], f32, tag="H", name="hpA")
                hpB = hpsum.tile([N, HALF], f32, tag="H", name="hpB")
                lhs = vT_s[:, t * N:(t + 1) * N]
                nc.tensor.matmul(hpA[:, 0:512], lhs, rh[:, 0:512],
                                 start=True, stop=True)
                nc.tensor.matmul(hpA[:, 512:1024], lhs, rh[:, 512:1024],
                                 start=True, stop=True)
                nc.tensor.matmul(hpB[:, 0:512], lhs, rh[:, 1024:1536],
                                 start=True, stop=True)
                nc.tensor.matmul(hpB[:, 512:1024], lhs, rh[:, 1536:2048],
                                 start=True, stop=True)

                if g2 == 0:
                    pd = dpsum.tile([N, 2 * N], f32, tag="D")
                nc.tensor.matmul(pd[:, g2 * N:(g2 + 1) * N],
                                 lhsTd_s[:, t * N:(t + 1) * N],
                                 rhsd_s[:, t * N:(t + 1) * N],
                                 start=True, stop=True)

                # evacuate into the 8-t batched R tile (signed bf16)
                if g8 == 0:
                    R8 = rwork.tile([N, TG * NA], bfl, tag="R8")
                R4 = R8[:].rearrange("p (g i a) -> p g i a", g=TG, a=A)
                for h, hp in ((0, hpA), (1, hpB)):
                    h3 = hp[:].rearrange("p (i a) -> p i a", a=A)
                    i0 = h * (N // 2)
                    i1 = (h + 1) * (N // 2)
                    Rh = R4[:, g8, i0:i1]
                    # ACT region: A-half ACT channels + B-half ACT channels.
                    # Contiguous only if the A half is fully ACT; otherwise
                    # two ACT instructions.
                    if a_max == 0 and a_min == 0:
                        if a_act + b_act > 0:
                            nc.scalar.activation(Rh[:, :, 0:8 + b_act],
                                                 h3[:, :, 0:8 + b_act],
                                                 Act.Relu)
                    else:
                        if a_act > 0:
                            nc.scalar.activation(Rh[:, :, 0:a_act],
                                                 h3[:, :, 0:a_act], Act.Relu)
                        if b_act > 0:
                            nc.scalar.activation(Rh[:, :, 8:8 + b_act],
                                                 h3[:, :, 8:8 + b_act],
                                                 Act.Relu)
                        if a_max > 0:
                            o = a_act
                            nc.vector.tensor_scalar(
                                Rh[:, :, o:o + a_max], h3[:, :, o:o + a_max],
                                0.0, None, op0=Alu.max)
                        if a_min > 0:
                            o = a_act + a_max
                            nc.vector.tensor_scalar(
                                Rh[:, :, o:o + a_min], h3[:, :, o:o + a_min],
                                0.0, None, op0=Alu.min)
                    if b_max > 0:
                        o = 8 + b_act
                        nc.vector.tensor_scalar(
                            Rh[:, :, o:o + b_max], h3[:, :, o:o + b_max],
                            0.0, None, op0=Alu.max)
                    if b_min > 0:
                        o = 8 + b_act + b_max
                        nc.vector.tensor_scalar(
                            Rh[:, :, o:o + b_min], h3[:, :, o:o + b_min],
                            0.0, None, op0=Alu.min)

                # per-2t soft mask (dist psum bank is single-buffered)
                if g2 == 1:
                    if g8 == 1:
                        att8 = awork.tile([N, TG * N], fp16, tag="att8")
                        m8 = awork.tile([N, TG * N], fp16, tag="m8")
                        w8 = awork.tile([N, TG * N], fp16, tag="w8")
                    nc.vector.tensor_scalar(
                        m8[:, (g8 - 1) * N:(g8 + 1) * N], pd[:], 0.0, 1.0,
                        op0=Alu.max, op1=Alu.min)

                if g8 == TG - 1:
                    # batched tree over the 8-t group
                    with nc.allow_low_precision(reason="bf16 channel sum"):
                        S1 = swork.tile([N, TG * NA // 2], bfl, tag="S1")
                        S14 = S1[:].rearrange("p (g i a) -> p g i a",
                                              g=TG, a=8)
                        nc.vector.tensor_tensor(S14[:], R4[:, :, :, 0:8],
                                                R4[:, :, :, 8:16],
                                                op=Alu.subtract)
                        S2 = swork.tile([N, TG * NA // 4], bfl, tag="S2")
                        S24 = S2[:].rearrange("p (g i a) -> p g i a",
                                              g=TG, a=4)
                        nc.vector.tensor_tensor(S24[:], S14[:, :, :, 0:4],
                                                S14[:, :, :, 4:8], op=Alu.add)
                        S3 = swork.tile([N, TG * NA // 8], bfl, tag="S3")
                        S34 = S3[:].rearrange("p (g i a) -> p g i a",
                                              g=TG, a=2)
                        nc.gpsimd.tensor_tensor(S34[:], S24[:, :, :, 0:2],
                                                S24[:, :, :, 2:4], op=Alu.add)
                        S4 = swork.tile([N, TG * N], bfl, tag="S4")
                        nc.gpsimd.tensor_tensor(
                            S4[:].rearrange("p (g i) -> p g i", g=TG),
                            S34[:, :, :, 0], S34[:, :, :, 1], op=Alu.add)

                    nc.scalar.activation(att8[:], S4[:], Act.Sigmoid,
                                         bias=b2val, scale=1.0)
                    nc.gpsimd.tensor_mul(w8[:], att8[:], m8[:])
                    if debug and t == TG - 1:
                        nc.sync.dma_start(dbg_m[:], m8[:])
                        nc.sync.dma_start(dbg_att[:], att8[:])
                        nc.sync.dma_start(dbg_s4[:], S4[:])
                        nc.sync.dma_start(dbg_r[:], R8[:, 0:NA])

                    pf = fpsum.tile([N, 4 * TG], f32, tag="F")
                    for gg in range(TG):
                        tt = t - (TG - 1) + gg
                        s = gg * N
                        nc.tensor.matmul(pf[:, 4 * gg:4 * gg + 3],
                                         w8[:, s:s + N],
                                         pos3_s[:, 3 * tt:3 * tt + 3],
                                         start=True, stop=True)
                        nc.tensor.matmul(pf[:, 4 * gg + 3:4 * gg + 4],
                                         m8[:, s:s + N],
                                         pos3_s[:, 3 * tt + 2:3 * tt + 3],
                                         start=True, stop=True)

                    # tail: out[i,c] = (num_c - pos_i_c*sum_w)/(cnt-1)
                    pf3 = pf[:].rearrange("p (g c) -> p g c", c=4)
                    pI3 = posI_s[:, 2 * (t - 7):2 * (t + 1)].rearrange(
                        "p (g c) -> p g c", c=2)
                    cnt8 = twork.tile([N, 8], f32, tag="cnt8")
                    rcp8 = twork.tile([N, 8], f32, tag="rcp8")
                    sw8 = twork.tile([N, 16], f32, tag="sw8")
                    outst = twork.tile([N, 16], f32, tag="outst")
                    nc.vector.tensor_scalar(cnt8[:], pf3[:, :, 3], -1.0, 1e-6,
                                            op0=Alu.add, op1=Alu.max)
                    nc.vector.reciprocal(rcp8[:], cnt8[:])
                    s3 = sw8[:].rearrange("p (g c) -> p g c", c=2)
                    o3 = outst[:].rearrange("p (g c) -> p g c", c=2)
                    for c in range(2):
                        nc.vector.tensor_mul(s3[:, :, c], pf3[:, :, 2],
                                             pI3[:, :, c])
                        nc.vector.tensor_sub(o3[:, :, c], pf3[:, :, c],
                                             s3[:, :, c])
                        nc.vector.tensor_mul(o3[:, :, c], o3[:, :, c],
                                             rcp8[:])
                    nc.sync.dma_start(
                        out_p[t - 7:t + 1].rearrange("t n c -> n t c"),
                        outst[:])

    nc.compile()
    return nc


def kernel(positions, W1, b1, W2, b2, _trace=False, _trace_kwargs=None):
    from concourse.bass_utils import run_bass_kernel_spmd

    prep = _host_prep(positions, W1, b1, W2, b2)
    plan = prep["plan"]
    b2v = prep["b2"]
    plan_key = (plan["a_act"], plan["a_max"], plan["a_min"],
                plan["b_act"], plan["b_max"], plan["b_min"])

    key = (plan_key, b2v)
    if key not in _CACHE:
        _CACHE[key] = _build_program(plan_key, b2v)
    nc = _CACHE[key]

    in_maps = []
    for b in range(B):
        in_maps.append({
            "vT": np.ascontiguousarray(prep["vT"][b]),
            "uflat": np.ascontiguousarray(prep["uflat"][b]),
            "delta": prep["delta"],
            "lhsTd": np.ascontiguousarray(prep["lhsTd"][b]),
            "rhsd": np.ascontiguousarray(prep["rhsd"][b]),
            "pos3": np.ascontiguousarray(prep["pos3"][b]),
            "posI": np.ascontiguousarray(prep["posI"][b]),
        })

    kw = {}
    if _trace:
        kw["trace"] = True
        if _trace_kwargs:
            kw.update(_trace_kwargs)
    res = run_bass_kernel_spmd(nc, in_maps, list(range(B)), **kw)
    out = np.stack([r["out"] for r in res.results], axis=0).astype(np.float32)
    if _trace:
        return out, res
    return out
